# revision 2
# baseline (speedup 1.0000x reference)
"""GatedDeltaNetCell kernel for 8 trn2 NeuronCores.

Strategy (per sharding hint): data-parallel over batch B=8 -> one batch
element per core. The T=1024 recurrence is reformulated chunkwise
(chunk C=64) via the WY representation so nearly all work is dense
matmul instead of 1024 sequential rank-1 updates:

  per chunk (local index i=1..C, per head):
    S_i = a_i S_{i-1} (I - b_i k_i k_i^T) + b_i v_i k_i^T
  With A_i = prod_{s<=i} a_s, Hh_i = I - b_i k_i k_i^T:
    (I + diag(b) L) W = diag(b) K      L = strict_tril(K K^T)
    (I + diag(b) L) U = diag(b/A) V
    O  = Qb S0^T + tril(Qb K^T) (U - W S0^T),   Qb = diag(A) Q
    S' = A_C (S0 (I - W^T K) + U^T K)
  The unit-triangular inverse is exact via Neumann doubling
  (nilpotent strictly-lower part): (I+N)^-1 = prod_k (I + (-N)^{2^k}).

Everything is fp32. Runs via jax on the 8 neuron cores (pmap over B);
falls back to CPU vmap if the accelerator path is unavailable.
"""

import numpy as np

T, B, D = 1024, 8, 1024
H, DH = 16, 64
C = 64            # chunk size
NC = T // C       # chunks
EPS = 1e-12


def _build_fn(jnp, jax):
    def l2norm(v):
        n = jnp.sqrt(jnp.sum(v * v, axis=-1, keepdims=True))
        return v / jnp.maximum(n, EPS)

    def per_batch(x, S0, W_q, W_k, W_v, W_alpha, b_alpha, W_beta, b_beta):
        # x: [T, D], S0: [H, DH, DH]
        q = l2norm((x @ W_q.T).reshape(T, H, DH))
        k = l2norm((x @ W_k.T).reshape(T, H, DH))
        v = (x @ W_v.T).reshape(T, H, DH)
        alpha = jax.nn.sigmoid(x @ W_alpha.T + b_alpha).reshape(T, H, DH).mean(-1)
        beta = jax.nn.sigmoid(x @ W_beta.T + b_beta).reshape(T, H, DH).mean(-1)

        # -> [NC, H, C, DH] / [NC, H, C]
        qc = q.reshape(NC, C, H, DH).transpose(0, 2, 1, 3)
        kc = k.reshape(NC, C, H, DH).transpose(0, 2, 1, 3)
        vc = v.reshape(NC, C, H, DH).transpose(0, 2, 1, 3)
        ac = alpha.reshape(NC, C, H).transpose(0, 2, 1)
        bc = beta.reshape(NC, C, H).transpose(0, 2, 1)

        A = jnp.cumprod(ac, axis=-1)                      # [NC, H, C]
        G = jnp.einsum('nhie,nhje->nhij', kc, kc)         # [NC, H, C, C]
        tril_s = jnp.tril(jnp.ones((C, C), jnp.float32), -1)
        N = G * tril_s * bc[..., :, None]                 # strictly lower, rows scaled by beta
        # (I+N)^-1 exactly (N nilpotent, index <= C): Neumann doubling
        eye = jnp.eye(C, dtype=jnp.float32)
        M = -N
        Tinv = eye + M
        for _ in range(5):                                # covers powers < 64
            M = M @ M
            Tinv = Tinv + Tinv @ M
        Kt = bc[..., None] * kc                           # diag(b) K
        Vt = (bc / A)[..., None] * vc                     # diag(b/A) V
        W = Tinv @ Kt                                     # [NC, H, C, DH]
        U = Tinv @ Vt
        Qb = A[..., None] * qc
        P = jnp.einsum('nhie,nhje->nhij', Qb, kc) * jnp.tril(
            jnp.ones((C, C), jnp.float32))                # inclusive diag
        Qp = Qb - P @ W                                   # [NC, H, C, DH]
        PU = P @ U
        WtK = jnp.einsum('nhie,nhif->nhef', W, kc)        # W^T K  [NC,H,DH,DH]
        UtK = jnp.einsum('nhie,nhif->nhef', U, kc)        # U^T K
        Ac = A[..., -1]                                   # [NC, H]

        S = S0                                            # [H, DH, DH]
        outs = []
        for c in range(NC):
            St = jnp.swapaxes(S, -1, -2)                  # S^T [H, DH(e), DH(d)]
            O = jnp.einsum('hie,hed->hid', Qp[c], St) + PU[c]
            outs.append(O)                                # [H, C, DH]
            S = Ac[c][:, None, None] * (
                S - jnp.einsum('hde,hef->hdf', S, WtK[c]) + UtK[c])
        O_all = jnp.stack(outs, 0)                        # [NC, H, C, DH]
        out = O_all.transpose(0, 2, 1, 3).reshape(T, D)
        return out, S

    return per_batch


def kernel(x, S0, W_q, W_k, W_v, W_alpha, b_alpha, W_beta, b_beta):
    import jax
    import jax.numpy as jnp

    per_batch = _build_fn(jnp, jax)
    args = (np.asarray(x, np.float32), np.asarray(S0, np.float32))
    ws = tuple(np.asarray(w, np.float32)
               for w in (W_q, W_k, W_v, W_alpha, b_alpha, W_beta, b_beta))

    # x: [T, B, D] -> per-core [B, T, D]
    xb = np.ascontiguousarray(args[0].transpose(1, 0, 2))

    def run_devices():
        devs = jax.devices()
        if len(devs) < B:
            raise RuntimeError('need 8 devices')
        f = jax.pmap(per_batch,
                     in_axes=(0, 0) + (None,) * 7,
                     devices=devs[:B])
        out_b, S_b = f(xb, args[1], *ws)
        return np.asarray(out_b), np.asarray(S_b)

    try:
        out_b, S_b = run_devices()
    except Exception:
        f = jax.jit(jax.vmap(per_batch, in_axes=(0, 0) + (None,) * 7),
                    backend='cpu')
        out_b, S_b = f(xb, args[1], *ws)
        out_b, S_b = np.asarray(out_b), np.asarray(S_b)

    outs = np.ascontiguousarray(out_b.transpose(1, 0, 2)).astype(np.float32)
    S_final = np.ascontiguousarray(S_b).astype(np.float32)
    return outs, S_final
